# revision 27
# baseline (speedup 1.0000x reference)
"""Multi-head attention (B=64, N=577, E=1024, H=16) on 8 TRN2 NeuronCores.

Strategy: pure data-parallel over batch (8 batches/core), full weights on
every core. Per (batch, head): scores are computed in transposed
orientation S^T[nk, nq] so softmax needs no probability transpose; the
softmax denominator comes for free from a ones-column appended to V in the
P@V matmul; normalization + the final [b, n, e] permute happen on the host.

v7 (655us -> 572us local amortized HW):
- x^T and W^T via DMA xbar transpose (SBUF->SBUF, bf16): all transpose
  work moves off the PE (saves ~60k PE cycles/core + prologue PE time);
- prologue warmup matmuls on a zero tile bridge the first DMA latencies
  and ramp the PE p-state;
- W loads + transposes ride the ACT HWDGE queue, x loads the SP queue
  (parallel on HW);
- exp merged across the two heads of a pair: one ACT instruction per
  (chunk, k-block) over [k, 2, cw]; scores/exp chunked (512, 66), PV and
  proj chunked (290, 288) so every matmul stream stays >= 288 (matmul
  cost on HW is max(stream, stationary-load) — short streams lose);
- PV emission delayed one pair so exp latency hides behind the next
  pair's scores; next-batch proj emission is hand-paced so the last
  batch still has PE filler work;
- output stored bf16 (halves store DMA);
- PSUM: scores 2x[128,1024] + PV 2x[65,512] + proj 2x[128,512] = 8 banks.
"""

import numpy as np

B, N, E, H, D = 64, 577, 1024, 16, 64
NCORES = 8
BL = B // NCORES            # batches per core
NP = 578                    # padded nq (even; pad col is zeroed)
EB = E // 128               # 8 e-blocks
NBL = [(i * 128, min(128, N - i * 128)) for i in range((N + 127) // 128)]
NNB = len(NBL)                       # 5 n-blocks
SCHUNKS = [(0, 512), (512, 66)]      # scores/exp chunks (psum bank width)
PCHUNKS = [(0, 290), (290, 288)]     # PV chunks (streams >= 288)
# qk-proj chunks over block-major x^T: (first block, nblocks, width, dst col)
QCHUNKS = [(0, 2, 128, 0), (2, 2, 128, 256), (4, 1, 66, 512)]

_CACHE = {}


def _build(cfg=None):
    cfg = cfg or {}
    ST = cfg.get("st", 2)
    PV = cfg.get("pv", 2)
    MM = cfg.get("mm", 2)
    QKB = cfg.get("qkb", 10)  # qt/kt rotation depth (pairs in flight)
    ESB = cfg.get("esb", 14)
    OVB = cfg.get("ovb", 4)
    WU = cfg.get("wu", 64)   # prologue PE warmup matmuls (p-state ramp)
    PACE = cfg.get("pace", 26)  # next-batch proj emission steps per pair
    import concourse.mybir as mybir
    import concourse.tile as tile
    from concourse import bacc
    from concourse.masks import make_identity

    f32 = mybir.dt.float32
    bf16 = mybir.dt.bfloat16
    Exp = mybir.ActivationFunctionType.Exp

    nc = bacc.Bacc("TRN2", target_bir_lowering=False, debug=False,
                   num_devices=NCORES)
    x = nc.declare_dram_parameter("x", [BL, N, E], f32, isOutput=False)
    Wq = nc.declare_dram_parameter("Wq", [E, E], f32, isOutput=False)
    Wk = nc.declare_dram_parameter("Wk", [E, E], f32, isOutput=False)
    Wv = nc.declare_dram_parameter("Wv", [E, E], f32, isOutput=False)
    # output in [b, head, d(+denom), n] layout, bf16; the host gather
    # applies the softmax normalization and the final [b, n, e] permute
    out = nc.declare_dram_parameter("out", [BL, H, D + 1, N], bf16,
                                    isOutput=True)

    with tile.TileContext(nc) as tc:
        with (
            tc.tile_pool(name="sb", bufs=1) as sb,
            tc.tile_pool(name="ps", bufs=1, space="PSUM") as ps,
        ):
            # ---- PE warmup: dummy matmuls on a memset-zero tile keep the
            # PE busy (and ramp its p-state) while the first weight/x DMAs
            # are in flight ----
            if WU:
                wz = sb.tile([128, 128], bf16, tag="wz", name="wz")
                nc.gpsimd.memset(wz[:, :], 0.0)
                warm = ps.tile([128, 1024], f32, tag="st", bufs=ST,
                               name="warm")
                for _ in range(WU):
                    nc.tensor.matmul(warm[:, :128], wz[:, :], wz[:, :],
                                     start=True, stop=True)

            ident = sb.tile([128, 128], bf16, tag="id", name="ident")
            make_identity(nc, ident[:])

            # ---- weights: W^T as one [e_in 128, ei 8, e_out 1024] tile
            # per matrix. W transposes run on the PE (prologue/batch-0
            # filler work); x transposes use the DMA xbar ----
            wt = {wi: sb.tile([128, EB, E], bf16, tag=f"wt{wi}",
                              name=f"wt{wi}") for wi in range(3)}

            def stage_cvt(W_dram, r0):
                # DMA a 128-row block of a f32 DRAM matrix (ACT queue,
                # parallel to the x loads on SP), convert to bf16
                ld = sb.tile([128, E], f32, tag="stage", bufs=3, name="ld")
                nc.scalar.dma_start(out=ld[:, :], in_=W_dram[r0:r0 + 128, :])
                bft = sb.tile([128, E], bf16, tag="bfst", bufs=3, name="bf")
                nc.vector.tensor_copy(bft[:, :], ld[:, :])
                return bft

            def tr_pack4(dst3d, src, s0, step=None):
                # transpose 4 consecutive 128-col blocks of `src` via plain
                # matmuls (out = block.T @ I) packed into ONE bf16 psum
                # tile, evacuated by ONE strided DVE copy.
                pt = ps.tile([128, 512], f32, tag="mm", bufs=MM, name="ptp")
                for s in range(4):
                    ei = s0 + s
                    nc.tensor.matmul(
                        pt[:, s * 128:(s + 1) * 128],
                        src[:, ei * 128:(ei + 1) * 128],
                        ident[:, :], start=True, stop=True)
                    if step:
                        yield None
                nc.vector.tensor_copy(
                    dst3d, pt[:, :].rearrange("p (s c) -> p s c", s=4))
                if step:
                    yield None

            def make_xtv(b):
                # x^T tiles for batch b + V->vext, as TWO generators:
                # gen_xtr (loads + xbar transposes; touches no weights) and
                # gen_vproj (V projection; reads wt[2]).
                # Block-major layout [p, nb, ei, 128]: each xbar transpose
                # lands in a fully contiguous [128, 1024] region (strided
                # xbar destinations corrupt silently).
                xt = sb.tile([128, NNB, EB, 128], bf16, tag="xt", bufs=2,
                             name="xt")
                # bufs=3: with PV delayed one pair, batch b-1's last PV
                # still reads vext(b-1) while the vproj filler of b+1
                # writes — 3 generations alive
                vext = [sb.tile([128, H, D + 1], bf16, tag=f"vx_{nb}",
                                bufs=3, name=f"vx{nb}")
                        for nb in range(len(NBL))]

                def gen_xtr():
                    for nb, (n0, nsz) in enumerate(NBL):
                        nc.gpsimd.memset(vext[nb][:nsz, :, D:D + 1], 1.0)
                    xbfs = []
                    for nb, (n0, nsz) in enumerate(NBL):
                        ld = sb.tile([128, E], f32, tag="xstage", bufs=5,
                                     name="xld")
                        if nsz < 128:
                            # zero the pad rows -> zero pad col 577 (and
                            # harmless zero cols 578-639) after transpose.
                            # gpsimd needs 32-aligned partitions: zero
                            # 64:128 first, the DMA then refills row 64.
                            nc.gpsimd.memset(ld[64:128, :], 0.0)
                        nc.sync.dma_start(out=ld[:nsz, :],
                                          in_=x[b, n0:n0 + nsz, :])
                        xbf = sb.tile([128, E], bf16, tag="xbfst", bufs=5,
                                      name="xbf")
                        nc.vector.tensor_copy(xbf[:, :], ld[:, :])
                        xbfs.append(xbf)
                        yield None
                    for nb, xbf in enumerate(xbfs):
                        nc.sync.dma_start_transpose(
                            xt[:, nb, :, :], xbf[:, :])
                        yield None

                def gen_vproj():
                    for nb, (n0, nsz) in enumerate(NBL):
                        for ec in range(2):
                            pv = ps.tile([128, 512], f32, tag="mm", bufs=MM,
                                         name="pv")
                            for ei in range(EB):
                                nc.tensor.matmul(
                                    pv[:nsz, :], xt[:, nb, ei, :nsz],
                                    wt[2][:, ei, ec * 512:(ec + 1) * 512],
                                    start=(ei == 0), stop=(ei == EB - 1))
                                yield None
                            nc.vector.tensor_copy(
                                vext[nb][:nsz, ec * 8:(ec + 1) * 8, 0:D],
                                pv[:nsz, :].rearrange("p (h d) -> p h d", d=D))
                            yield None

                return xt, vext, gen_xtr(), gen_vproj()

            def emit_qk(j, xt, with_w):
                qt = sb.tile([128, NP], bf16, tag="qt", bufs=QKB, name="qt")
                kt = sb.tile([128, NP], bf16, tag="kt", bufs=QKB, name="kt")

                def gen():
                    if with_w:
                        # transpose the eo=j block of Wq/Wk just-in-time
                        for wi, W in ((0, Wq), (1, Wk)):
                            wbf = stage_cvt(W, j * 128)
                            for s0 in (0, 4):
                                yield from tr_pack4(
                                    wt[wi][:, s0:s0 + 4,
                                           j * 128:(j + 1) * 128],
                                    wbf, s0, step=True)
                    for dst, wi in ((qt, 0), (kt, 1)):
                        for nb0, nnb, w, d0 in QCHUNKS:
                            cw = nnb * w
                            pq = ps.tile([128, 512], f32, tag="mm", bufs=MM,
                                         name="pq")
                            for ei in range(EB):
                                nc.tensor.matmul(
                                    pq[:, :cw],
                                    wt[wi][:, ei, j * 128:(j + 1) * 128],
                                    xt[:, nb0:nb0 + nnb, ei, :w],
                                    start=(ei == 0), stop=(ei == EB - 1))
                                yield None
                            nc.vector.tensor_copy(
                                dst[:, d0:d0 + cw], pq[:, :cw])
                            yield None

                return qt, kt, gen()

            # ---- filler machinery: one ordered queue of generators ----
            fillers = []

            def fill(n):
                for _ in range(n):
                    if not fillers:
                        return
                    for it in list(fillers):
                        if next(it, StopIteration) is StopIteration:
                            fillers.remove(it)
                        else:
                            break

            def drain(it, n=10 ** 6):
                for _ in range(n):
                    if next(it, StopIteration) is StopIteration:
                        return

            registry = {}
            bundles = {}

            def proj_batch(b, pieces=None):
                xt, vext, gx, gv = pieces if pieces else make_xtv(b)
                bundles[b] = (xt, vext)

                def gen():
                    yield from gx
                    yield from gv
                    for j in range(H // 2):
                        qt, kt, qg = emit_qk(j, xt, with_w=(b == 0))
                        registry[b, j] = (qt, kt)
                        yield from qg
                        registry[b, j, "done"] = True
                return gen()

            # ---- weight prologue: Wv staged (ACT queue) + PE-transposed
            # first (V proj reads it); a few x-block loads interleave so
            # batch 0's xbar transposes overlap; rest of gx0 resumes
            # inside proj_batch(0) ----
            pieces0 = make_xtv(0)
            gx0 = pieces0[2]
            for eo in range(EB):
                wbf = stage_cvt(Wv, eo * 128)
                if eo in (1, 3, 5):
                    drain(gx0, 1)
                for s0 in (0, 4):
                    for _ in tr_pack4(wt[2][:, s0:s0 + 4,
                                            eo * 128:(eo + 1) * 128],
                                      wbf, s0, step=True):
                        pass

            proj_gen = {0: proj_batch(0, pieces=pieces0)}
            fillers.append(proj_gen[0])

            def emit_pv(b, j, es, vext):
                # --- PV: stationary = V[+ones] k-block, moving = exp(S^T)
                # chunk; out [65, cw] accumulated over k-blocks. Emitted
                # one pair late so exp latency hides behind the next
                # pair's scores. ---
                ov = {}
                for ci, (c0, cw) in enumerate(PCHUNKS):
                    for h in range(2):
                        if ci == 0:
                            ov[h] = sb.tile([D + 1, NP], bf16, tag="ov",
                                            bufs=OVB, name="ov")
                        pO = ps.tile([D + 1, 512], f32, tag="pv", bufs=PV,
                                     name="pO")
                        for kb, (k0, ksz) in enumerate(NBL):
                            nc.tensor.matmul(
                                pO[:, :cw],
                                vext[kb][:ksz, 2 * j + h, :],
                                es[kb][:ksz, h, c0:c0 + cw],
                                start=(kb == 0),
                                stop=(kb == len(NBL) - 1))
                            fill(1)
                        nc.vector.tensor_copy(
                            ov[h][:, c0:c0 + cw], pO[:, :cw])
                        fill(1)
                        if ci == len(PCHUNKS) - 1:
                            nc.gpsimd.dma_start(
                                out=out[b, 2 * j + h, :, :],
                                in_=ov[h][:, :N])
                            fill(1)

            pending = None
            for b in range(BL):
                # next batch's proj is PACED (manual per-pair drains), not
                # a free-running filler: over-emitting it during batch b
                # leaves batch b+1's attention with no PE filler work
                if b + 1 < BL:
                    proj_gen[b + 1] = proj_batch(b + 1)
                if b < BL - 1 and proj_gen[b] not in fillers:
                    fillers.append(proj_gen[b])
                xt_cur, vext = bundles[b]

                for j in range(H // 2):
                    # force pair (b, j) emission to completion
                    while (b, j, "done") not in registry:
                        drain(proj_gen[b], 1)
                    if b + 1 < BL:
                        drain(proj_gen[b + 1], PACE)
                        if j == H // 2 - 1 and b + 1 < BL - 1:
                            # own proj exhausted past here; let the next
                            # batch's proj free-run as filler. The LAST
                            # batch's proj instead stays hand-paced so it
                            # lasts through the whole final batch.
                            fillers.append(proj_gen[b + 1])
                    else:
                        # last batch: spread own remaining proj emission
                        drain(proj_gen[b], PACE)
                    qt, kt = registry[b, j]

                    # --- scores + exp: S^T[k, q] per k-block, both heads
                    # side by side in one psum tile; one ACT exp per
                    # (chunk, k-block) covering [k, 2, cw] -> es ---
                    es = [sb.tile([128, 2, NP], bf16, tag="es", bufs=ESB,
                                  name="es") for _ in range(len(NBL))]
                    for c0, cw in SCHUNKS:
                        for kb, (k0, ksz) in enumerate(NBL):
                            pS = ps.tile([128, 1024], f32, tag="st",
                                         bufs=ST, name="pS")
                            for h in range(2):
                                nc.tensor.matmul(
                                    pS[:ksz, h * 512:h * 512 + cw],
                                    kt[h * 64:h * 64 + 64, k0:k0 + ksz],
                                    qt[h * 64:h * 64 + 64, c0:c0 + cw],
                                    start=True, stop=True,
                                    tile_position=(h * 64, 0))
                                fill(1)
                            src = pS[:ksz, :].rearrange(
                                "p (h c) -> p h c", h=2)
                            nc.scalar.activation(
                                es[kb][:ksz, :, c0:c0 + cw],
                                src[:, :, :cw], Exp, scale=0.125)
                            fill(1)

                    if pending is not None:
                        emit_pv(*pending)
                    pending = (b, j, es, vext)
            emit_pv(*pending)
            fill(10 ** 6)

    nc.compile()
    return nc


def in_maps_for_bench(inputs):
    x = np.ascontiguousarray(np.asarray(inputs["x"], dtype=np.float32))
    Wq = np.ascontiguousarray(np.asarray(inputs["Wq"], dtype=np.float32))
    Wk = np.ascontiguousarray(np.asarray(inputs["Wk"], dtype=np.float32))
    Wv = np.ascontiguousarray(np.asarray(inputs["Wv"], dtype=np.float32))
    xs = x.reshape(NCORES, BL, N, E)
    return [
        {"x": np.ascontiguousarray(xs[i]), "Wq": Wq, "Wk": Wk, "Wv": Wv}
        for i in range(NCORES)
    ]


def kernel(x, Wq, Wk, Wv):
    from concourse.bass_utils import run_bass_kernel_spmd

    if "nc" not in _CACHE:
        _CACHE["nc"] = _build()
    nc = _CACHE["nc"]

    in_maps = in_maps_for_bench({"x": x, "Wq": Wq, "Wk": Wk, "Wv": Wv})
    res = run_bass_kernel_spmd(nc, in_maps, core_ids=list(range(NCORES)))
    # device emits [b, head, d(+denom), n] bf16; normalize + permute here
    ot = np.concatenate([res.results[i]["out"] for i in range(NCORES)],
                        axis=0).astype(np.float32)
    o = ot[:, :, :D, :] / ot[:, :, D:D + 1, :]
    return np.ascontiguousarray(
        o.transpose(0, 3, 1, 2).reshape(B, N, E).astype(np.float32))


# revision 32
# speedup vs baseline: 1.0558x; 1.0558x over previous
"""Multi-head attention (B=64, N=577, E=1024, H=16) on 8 TRN2 NeuronCores.

Strategy: pure data-parallel over batch (8 batches/core), full weights on
every core. Per (batch, head): scores are computed in transposed
orientation S^T[nk, nq] so softmax needs no probability transpose; the
softmax denominator comes for free from a ones-column appended to V in the
P@V matmul; normalization + the final [b, n, e] permute happen on the host.

v7 (655us -> 572us local amortized HW):
- x^T and W^T via DMA xbar transpose (SBUF->SBUF, bf16): all transpose
  work moves off the PE (saves ~60k PE cycles/core + prologue PE time);
- prologue warmup matmuls on a zero tile bridge the first DMA latencies
  and ramp the PE p-state;
- W loads + transposes ride the ACT HWDGE queue, x loads the SP queue
  (parallel on HW);
- exp merged across the two heads of a pair: one ACT instruction per
  (chunk, k-block) over [k, 2, cw]; scores/exp chunked (512, 66), PV and
  proj chunked (290, 288) so every matmul stream stays >= 288 (matmul
  cost on HW is max(stream, stationary-load) — short streams lose);
- PV emission delayed one pair so exp latency hides behind the next
  pair's scores; next-batch proj emission is hand-paced so the last
  batch still has PE filler work;
- output stored bf16 (halves store DMA);
- PSUM: scores 2x[128,1024] + PV 2x[65,512] + proj 2x[128,512] = 8 banks.
"""

import numpy as np

B, N, E, H, D = 64, 577, 1024, 16, 64
NCORES = 8
BL = B // NCORES            # batches per core
NP = 578                    # padded nq (even; pad col is zeroed)
EB = E // 128               # 8 e-blocks
NBL = [(i * 128, min(128, N - i * 128)) for i in range((N + 127) // 128)]
NNB = len(NBL)                       # 5 n-blocks
SCHUNKS = [(0, 512), (512, 66)]      # scores/exp chunks (psum bank width)
PCHUNKS = [(0, 290), (290, 288)]     # PV chunks (streams >= 288)
# qk-proj chunks over block-major x^T: (first block, nblocks, width, dst col)
QCHUNKS = [(0, 2, 128, 0), (2, 2, 128, 256), (4, 1, 66, 512)]

_CACHE = {}


def _build(cfg=None):
    cfg = cfg or {}
    ST = cfg.get("st", 2)
    PV = cfg.get("pv", 2)
    MM = cfg.get("mm", 2)
    QKB = cfg.get("qkb", 10)  # qt/kt rotation depth (pairs in flight)
    ESB = cfg.get("esb", 14)
    OVB = cfg.get("ovb", 4)
    WU = cfg.get("wu", 64)   # prologue PE warmup matmuls (p-state ramp)
    NOXT = cfg.get("noxt", 0)  # timing probe: skip x transposes (WRONG results)
    XPE = cfg.get("xpe", 0)  # x transposes on PE (tr_pack4) instead of xbar
    PACE = cfg.get("pace", 26)  # next-batch proj emission steps per pair
    import concourse.mybir as mybir
    import concourse.tile as tile
    from concourse import bacc
    from concourse.masks import make_identity

    f32 = mybir.dt.float32
    bf16 = mybir.dt.bfloat16
    Exp = mybir.ActivationFunctionType.Exp

    nc = bacc.Bacc("TRN2", target_bir_lowering=False, debug=False,
                   num_devices=NCORES)
    x = nc.declare_dram_parameter("x", [BL, N, E], f32, isOutput=False)
    Wq = nc.declare_dram_parameter("Wq", [E, E], f32, isOutput=False)
    Wk = nc.declare_dram_parameter("Wk", [E, E], f32, isOutput=False)
    Wv = nc.declare_dram_parameter("Wv", [E, E], f32, isOutput=False)
    # output in [b, head, d(+denom), n] layout, bf16; the host gather
    # applies the softmax normalization and the final [b, n, e] permute
    out = nc.declare_dram_parameter("out", [BL, H, D + 1, N], bf16,
                                    isOutput=True)

    with tile.TileContext(nc) as tc:
        with (
            tc.tile_pool(name="sb", bufs=1) as sb,
            tc.tile_pool(name="ps", bufs=1, space="PSUM") as ps,
        ):
            # ---- PE warmup: dummy matmuls on a memset-zero tile keep the
            # PE busy (and ramp its p-state) while the first weight/x DMAs
            # are in flight ----
            if WU:
                wz = sb.tile([128, 128], bf16, tag="wz", name="wz")
                nc.gpsimd.memset(wz[:, :], 0.0)
                warm = ps.tile([128, 1024], f32, tag="st", bufs=ST,
                               name="warm")
                for _ in range(WU):
                    nc.tensor.matmul(warm[:, :128], wz[:, :], wz[:, :],
                                     start=True, stop=True)

            ident = sb.tile([128, 128], bf16, tag="id", name="ident")
            make_identity(nc, ident[:])

            # ---- weights: W^T as one [e_in 128, ei 8, e_out 1024] tile
            # per matrix. W transposes run on the PE (prologue/batch-0
            # filler work); x transposes use the DMA xbar ----
            wt = {wi: sb.tile([128, EB, E], bf16, tag=f"wt{wi}",
                              name=f"wt{wi}") for wi in range(3)}

            def stage_cvt(W_dram, r0):
                # DMA a 128-row block of a f32 DRAM matrix (ACT queue,
                # parallel to the x loads on SP), convert to bf16
                ld = sb.tile([128, E], f32, tag="stage", bufs=3, name="ld")
                nc.scalar.dma_start(out=ld[:, :], in_=W_dram[r0:r0 + 128, :])
                bft = sb.tile([128, E], bf16, tag="bfst", bufs=3, name="bf")
                nc.vector.tensor_copy(bft[:, :], ld[:, :])
                return bft

            def tr_pack4(dst3d, src, s0, step=None):
                # transpose 4 consecutive 128-col blocks of `src` via plain
                # matmuls (out = block.T @ I) packed into ONE bf16 psum
                # tile, evacuated by ONE strided DVE copy.
                pt = ps.tile([128, 512], f32, tag="mm", bufs=MM, name="ptp")
                for s in range(4):
                    ei = s0 + s
                    nc.tensor.matmul(
                        pt[:, s * 128:(s + 1) * 128],
                        src[:, ei * 128:(ei + 1) * 128],
                        ident[:, :], start=True, stop=True)
                    if step:
                        yield None
                nc.vector.tensor_copy(
                    dst3d, pt[:, :].rearrange("p (s c) -> p s c", s=4))
                if step:
                    yield None

            def make_xtv(b):
                # x^T tiles for batch b + V->vext, as TWO generators:
                # gen_xtr (loads + xbar transposes; touches no weights) and
                # gen_vproj (V projection; reads wt[2]).
                # Block-major layout [p, nb, ei, 128]: each xbar transpose
                # lands in a fully contiguous [128, 1024] region (strided
                # xbar destinations corrupt silently).
                xt = sb.tile([128, NNB, EB, 128], bf16, tag="xt", bufs=2,
                             name="xt")
                # bufs=3: with PV delayed one pair, batch b-1's last PV
                # still reads vext(b-1) while the vproj filler of b+1
                # writes — 3 generations alive
                vext = [sb.tile([128, H, D + 1], bf16, tag=f"vx_{nb}",
                                bufs=3, name=f"vx{nb}")
                        for nb in range(len(NBL))]

                def gen_xtr():
                    if NOXT:
                        nc.gpsimd.memset(xt[:, :, :, :], 0.0)
                    for nb, (n0, nsz) in enumerate(NBL):
                        nc.gpsimd.memset(vext[nb][:nsz, :, D:D + 1], 1.0)
                    xbfs = []
                    for nb, (n0, nsz) in enumerate(NBL):
                        ld = sb.tile([128, E], f32, tag="xstage", bufs=5,
                                     name="xld")
                        if nsz < 128:
                            # zero the pad rows -> zero pad col 577 (and
                            # harmless zero cols 578-639) after transpose.
                            # gpsimd needs 32-aligned partitions: zero
                            # 64:128 first, the DMA then refills row 64.
                            nc.gpsimd.memset(ld[64:128, :], 0.0)
                        nc.sync.dma_start(out=ld[:nsz, :],
                                          in_=x[b, n0:n0 + nsz, :])
                        xbf = sb.tile([128, E], bf16, tag="xbfst", bufs=5,
                                      name="xbf")
                        nc.vector.tensor_copy(xbf[:, :], ld[:, :])
                        xbfs.append(xbf)
                        yield None
                    for nb, xbf in enumerate(xbfs):
                        if NOXT:
                            yield None
                        elif XPE:
                            for s0 in (0, 4):
                                yield from tr_pack4(
                                    xt[:, nb, s0:s0 + 4, :], xbf, s0,
                                    step=True)
                        else:
                            nc.sync.dma_start_transpose(
                                xt[:, nb, :, :], xbf[:, :])
                            yield None

                def gen_vproj():
                    for nb, (n0, nsz) in enumerate(NBL):
                        for ec in range(2):
                            pv = ps.tile([128, 512], f32, tag="mm", bufs=MM,
                                         name="pv")
                            for ei in range(EB):
                                nc.tensor.matmul(
                                    pv[:nsz, :], xt[:, nb, ei, :nsz],
                                    wt[2][:, ei, ec * 512:(ec + 1) * 512],
                                    start=(ei == 0), stop=(ei == EB - 1))
                                yield None
                            nc.vector.tensor_copy(
                                vext[nb][:nsz, ec * 8:(ec + 1) * 8, 0:D],
                                pv[:nsz, :].rearrange("p (h d) -> p h d", d=D))
                            yield None

                return xt, vext, gen_xtr(), gen_vproj()

            def emit_qk(j, xt, with_w):
                qt = sb.tile([128, NP], bf16, tag="qt", bufs=QKB, name="qt")
                kt = sb.tile([128, NP], bf16, tag="kt", bufs=QKB, name="kt")

                def gen():
                    if with_w:
                        # transpose the eo=j block of Wq/Wk just-in-time
                        for wi, W in ((0, Wq), (1, Wk)):
                            wbf = stage_cvt(W, j * 128)
                            for s0 in (0, 4):
                                yield from tr_pack4(
                                    wt[wi][:, s0:s0 + 4,
                                           j * 128:(j + 1) * 128],
                                    wbf, s0, step=True)
                    for dst, wi in ((qt, 0), (kt, 1)):
                        for nb0, nnb, w, d0 in QCHUNKS:
                            cw = nnb * w
                            pq = ps.tile([128, 512], f32, tag="mm", bufs=MM,
                                         name="pq")
                            for ei in range(EB):
                                nc.tensor.matmul(
                                    pq[:, :cw],
                                    wt[wi][:, ei, j * 128:(j + 1) * 128],
                                    xt[:, nb0:nb0 + nnb, ei, :w],
                                    start=(ei == 0), stop=(ei == EB - 1))
                                yield None
                            nc.vector.tensor_copy(
                                dst[:, d0:d0 + cw], pq[:, :cw])
                            yield None

                return qt, kt, gen()

            # ---- filler machinery: one ordered queue of generators ----
            fillers = []

            def fill(n):
                for _ in range(n):
                    if not fillers:
                        return
                    for it in list(fillers):
                        if next(it, StopIteration) is StopIteration:
                            fillers.remove(it)
                        else:
                            break

            def drain(it, n=10 ** 6):
                for _ in range(n):
                    if next(it, StopIteration) is StopIteration:
                        return

            registry = {}
            bundles = {}

            def proj_batch(b, pieces=None):
                xt, vext, gx, gv = pieces if pieces else make_xtv(b)
                bundles[b] = (xt, vext)

                def gen():
                    yield from gx
                    yield from gv
                    for j in range(H // 2):
                        qt, kt, qg = emit_qk(j, xt, with_w=(b == 0))
                        registry[b, j] = (qt, kt)
                        yield from qg
                        registry[b, j, "done"] = True
                return gen()

            # ---- weight prologue: Wv staged (ACT queue) + PE-transposed
            # first (V proj reads it); a few x-block loads interleave so
            # batch 0's xbar transposes overlap; rest of gx0 resumes
            # inside proj_batch(0) ----
            pieces0 = make_xtv(0)
            gx0 = pieces0[2]
            for eo in range(EB):
                wbf = stage_cvt(Wv, eo * 128)
                if eo in (1, 3, 5):
                    drain(gx0, 1)
                for s0 in (0, 4):
                    for _ in tr_pack4(wt[2][:, s0:s0 + 4,
                                            eo * 128:(eo + 1) * 128],
                                      wbf, s0, step=True):
                        pass

            proj_gen = {0: proj_batch(0, pieces=pieces0)}
            fillers.append(proj_gen[0])

            def emit_pv(b, j, es, vext):
                # --- PV: stationary = V[+ones] k-block, moving = exp(S^T)
                # chunk; out [65, cw] accumulated over k-blocks. Emitted
                # one pair late so exp latency hides behind the next
                # pair's scores. ---
                ov = {}
                for ci, (c0, cw) in enumerate(PCHUNKS):
                    for h in range(2):
                        if ci == 0:
                            ov[h] = sb.tile([D + 1, NP], bf16, tag="ov",
                                            bufs=OVB, name="ov")
                        pO = ps.tile([D + 1, 512], f32, tag="pv", bufs=PV,
                                     name="pO")
                        for kb, (k0, ksz) in enumerate(NBL):
                            nc.tensor.matmul(
                                pO[:, :cw],
                                vext[kb][:ksz, 2 * j + h, :],
                                es[kb][:ksz, h, c0:c0 + cw],
                                start=(kb == 0),
                                stop=(kb == len(NBL) - 1))
                            fill(1)
                        nc.vector.tensor_copy(
                            ov[h][:, c0:c0 + cw], pO[:, :cw])
                        fill(1)
                        if ci == len(PCHUNKS) - 1:
                            nc.gpsimd.dma_start(
                                out=out[b, 2 * j + h, :, :],
                                in_=ov[h][:, :N])
                            fill(1)

            pending = None
            for b in range(BL):
                # next batch's proj is PACED (manual per-pair drains), not
                # a free-running filler: over-emitting it during batch b
                # leaves batch b+1's attention with no PE filler work
                if b + 1 < BL:
                    proj_gen[b + 1] = proj_batch(b + 1)
                if b < BL - 1 and proj_gen[b] not in fillers:
                    fillers.append(proj_gen[b])
                xt_cur, vext = bundles[b]

                for j in range(H // 2):
                    # force pair (b, j) emission to completion
                    while (b, j, "done") not in registry:
                        drain(proj_gen[b], 1)
                    if b + 1 < BL:
                        drain(proj_gen[b + 1], PACE)
                        if j == H // 2 - 1 and b + 1 < BL - 1:
                            # own proj exhausted past here; let the next
                            # batch's proj free-run as filler. The LAST
                            # batch's proj instead stays hand-paced so it
                            # lasts through the whole final batch.
                            fillers.append(proj_gen[b + 1])
                    else:
                        # last batch: spread own remaining proj emission
                        drain(proj_gen[b], PACE)
                    qt, kt = registry[b, j]

                    # --- scores + exp: S^T[k, q] per k-block, both heads
                    # side by side in one psum tile; one ACT exp per
                    # (chunk, k-block) covering [k, 2, cw] -> es ---
                    es = [sb.tile([128, 2, NP], bf16, tag="es", bufs=ESB,
                                  name="es") for _ in range(len(NBL))]
                    for c0, cw in SCHUNKS:
                        for kb, (k0, ksz) in enumerate(NBL):
                            pS = ps.tile([128, 1024], f32, tag="st",
                                         bufs=ST, name="pS")
                            for h in range(2):
                                nc.tensor.matmul(
                                    pS[:ksz, h * 512:h * 512 + cw],
                                    kt[h * 64:h * 64 + 64, k0:k0 + ksz],
                                    qt[h * 64:h * 64 + 64, c0:c0 + cw],
                                    start=True, stop=True,
                                    tile_position=(h * 64, 0))
                                fill(1)
                            src = pS[:ksz, :].rearrange(
                                "p (h c) -> p h c", h=2)
                            nc.scalar.activation(
                                es[kb][:ksz, :, c0:c0 + cw],
                                src[:, :, :cw], Exp, scale=0.125)
                            fill(1)

                    if pending is not None:
                        emit_pv(*pending)
                    pending = (b, j, es, vext)
            emit_pv(*pending)
            fill(10 ** 6)

    nc.compile()
    return nc


def in_maps_for_bench(inputs):
    x = np.ascontiguousarray(np.asarray(inputs["x"], dtype=np.float32))
    Wq = np.ascontiguousarray(np.asarray(inputs["Wq"], dtype=np.float32))
    Wk = np.ascontiguousarray(np.asarray(inputs["Wk"], dtype=np.float32))
    Wv = np.ascontiguousarray(np.asarray(inputs["Wv"], dtype=np.float32))
    xs = x.reshape(NCORES, BL, N, E)
    return [
        {"x": np.ascontiguousarray(xs[i]), "Wq": Wq, "Wk": Wk, "Wv": Wv}
        for i in range(NCORES)
    ]


def kernel(x, Wq, Wk, Wv):
    from concourse.bass_utils import run_bass_kernel_spmd

    if "nc" not in _CACHE:
        _CACHE["nc"] = _build()
    nc = _CACHE["nc"]

    in_maps = in_maps_for_bench({"x": x, "Wq": Wq, "Wk": Wk, "Wv": Wv})
    res = run_bass_kernel_spmd(nc, in_maps, core_ids=list(range(NCORES)))
    # device emits [b, head, d(+denom), n] bf16; normalize + permute here
    ot = np.concatenate([res.results[i]["out"] for i in range(NCORES)],
                        axis=0).astype(np.float32)
    o = ot[:, :, :D, :] / ot[:, :, D:D + 1, :]
    return np.ascontiguousarray(
        o.transpose(0, 3, 1, 2).reshape(B, N, E).astype(np.float32))
